# revision 32
# baseline (speedup 1.0000x reference)
"""CRF log-likelihood kernel for Trainium2 (Bass/Tile), 8-core data parallel.

out[b] = gold_path_score(b) - logZ(b)

logZ via exp-domain DP with forward and backward chains meeting at t = F:
  fwd:  u_t   = el_t  ⊙ (Wf^T u_{t-1}),      t = 1..F      (u_0 = el_0)
  bwd:  γ_σ   = Wb^T (el_{T+1-σ} ⊙ γ_{σ-1}), σ = 1..T-F    (γ_0 = sink)
Sequences with len <= F finish inside the fwd chain via an absorbing "sink"
label that captures sum_i u_{len-1}[i] exactly at t == len; longer sequences
use the midpoint identity Z = Σ_j α_F[j]·β_F[j], with the bwd chain's sink
"birthing" β = 1 at each sequence's own end time. The two chains are
independent, so PE matmuls of one overlap DVE multiplies of the other.

Layout per core (128 sequences):
  partitions 0..95 = active labels (3 groups x 32), 96..98 = sink row per
  group; psum rows 99..101 = per-group column sums (ones-columns of the
  stationary operand). columns: b_local = 43*g + c.
Scaling: all emissions carry e^{-CSHIFT}; columns are renormalized by their
column sum mid-chain (factor tracked exactly via ACT-Ln of the applied
multiplier). Host adds CSHIFT*len back and picks sink vs combine per length.
Host also does the gold-path gathers (labels/trans only) and final subtract.
"""

import numpy as np
import ml_dtypes

B, T, L = 1024, 512, 32
NCORES = 8
BPC = B // NCORES        # 128 sequences per core
G = 3                    # label groups per core
NCOL = 43                # columns per group (group 2 uses 42 + 1 pad)
NACT = 96                # active label partitions
NPART = 99               # + 3 sink rows
MOUT = 102               # + 3 colsum rows
CSHIFT = 4.5
TEX = T + 1              # el time slices 0..T
F = 256                  # fwd ticks; bwd ticks = T - F
SB = T - F
RENORM_EVERY = 128


def _el_windows():
    """Graded (t0, n) windows covering [0, TEX), smallest first, alternating
    tail (bwd consumes from t=T down) and head (fwd from t=0 up) so both
    chains can start after ~2 small DMAs instead of waiting out 1MB chunks."""
    sizes = [4, 8, 16, 32, 64, 96]
    head, tail = [], []
    lo, hi = 0, TEX
    for s in sizes:
        tail.append((hi - s, s)); hi -= s
        head.append((lo, s)); lo += s
    # remainder split once more (tail gets the first share)
    rem = hi - lo
    a = rem // 2
    tail.append((hi - a, a)); hi -= a
    head.append((lo, hi - lo))
    order = []
    for tl, hd in zip(tail, head):
        order.append(tl); order.append(hd)
    return order


EL_WINDOWS = _el_windows()
STAGE_MAX = max(n for _, n in EL_WINDOWS)

_prog_cache = {}
last_result = None       # BassKernelResults of the most recent run (for test.py)


def _build_program():
    import concourse.bacc as bacc
    import concourse.tile as tile
    from concourse import mybir

    f32 = mybir.dt.float32
    bf16 = mybir.dt.bfloat16
    AF = mybir.ActivationFunctionType

    nc = bacc.Bacc("TRN2", target_bir_lowering=False, debug=False, num_devices=NCORES)
    lg = nc.dram_tensor("lg", [NACT, TEX, NCOL], f32, kind="ExternalInput")
    el32 = nc.dram_tensor("el32", [G, TEX, NCOL], bf16, kind="ExternalInput")
    wf = nc.dram_tensor("wf", [NPART, MOUT], bf16, kind="ExternalInput")
    wbk = nc.dram_tensor("wbk", [NPART, MOUT], bf16, kind="ExternalInput")
    wcs = nc.dram_tensor("wcs", [NPART, G], bf16, kind="ExternalInput")
    resf = nc.dram_tensor("resf", [G, NCOL], f32, kind="ExternalOutput")
    resc = nc.dram_tensor("resc", [G, NCOL], f32, kind="ExternalOutput")

    with tile.TileContext(nc) as tc:
        with (
            tc.tile_pool(name="big", bufs=1) as big,
            tc.tile_pool(name="stage", bufs=3) as stage_p,
            tc.tile_pool(name="consts", bufs=1) as consts,
            tc.tile_pool(name="u", bufs=3) as upool,
            tc.tile_pool(name="v", bufs=3) as vpool,
            tc.tile_pool(name="fin", bufs=1) as fin,
            tc.tile_pool(name="psf", bufs=4, space="PSUM") as psfpool,
        ):
            el_sb = big.tile([NPART, TEX, NCOL], bf16)
            wf_sb = consts.tile([NPART, MOUT], bf16)
            wb_sb = consts.tile([NPART, MOUT], bf16)
            wcs_sb = consts.tile([NPART, G], bf16)
            biasc = consts.tile([128, 1], f32)
            g0 = consts.tile([NPART, NCOL], bf16)
            nc.vector.memset(biasc[:], -CSHIFT)
            nc.vector.memset(g0[:], 0.0)
            nc.vector.memset(g0[NACT:NPART, :], 1.0)

            # warm the ACT Exp/Ln tables while the first DMAs are in flight
            warm = consts.tile([1, 2], f32)
            nc.scalar.activation(warm[:, 0:1], biasc[0:1, :], AF.Exp)
            nc.scalar.activation(warm[:, 1:2], warm[:, 0:1], AF.Ln)

            nc.sync.dma_start(out=wf_sb[:], in_=wf[:])
            nc.sync.dma_start(out=wb_sb[:], in_=wbk[:])
            nc.sync.dma_start(out=wcs_sb[:], in_=wcs[:])
            # active rows: stage raw logits, bulk-exp into el_sb.
            # graded windows, alternating ends: bwd consumes from t=T down.
            # sink rows (partitions 96..98) stream in the same windows so no
            # monolithic descriptor hogs a DMA engine ahead of the first chunks.
            for t0, n in EL_WINDOWS:
                st = stage_p.tile([NACT, STAGE_MAX, NCOL], f32, tag="stage")
                nc.sync.dma_start(out=st[:, 0:n, :], in_=lg[:, t0 : t0 + n, :])
                nc.sync.dma_start(
                    out=el_sb[NACT:NPART, t0 : t0 + n, :], in_=el32[:, t0 : t0 + n, :]
                )
                nc.scalar.activation(
                    el_sb[0:NACT, t0 : t0 + n, :],
                    st[:, 0:n, :],
                    AF.Exp,
                    bias=biasc[0:NACT, :],
                )

            # Four independent latency lanes: fwd/bwd x column halves.
            # No renorm: CSHIFT ~ log(L * E[e^trans] * E[e^logit]) makes the
            # expected per-tick log-drift ~0; the +-4 sigma random walk over
            # 256 ticks stays well inside bf16/f32 exponent range.
            CW = [(0, NCOL)]
            uprev = [el_sb[:, 0, c0:c1] for c0, c1 in CW]
            gprev = [g0[:, c0:c1] for c0, c1 in CW]
            gprev_sbuf = [True, True]
            ulast = [None, None]
            pb_last = [None, None]
            for k in range(1, max(F, SB) + 1):
                # ---- fwd tick t = k (both column halves) ----
                if k <= F:
                    psfs = []
                    for h, (c0, c1) in enumerate(CW):
                        psf = psfpool.tile([MOUT, c1 - c0], f32, tag="ps")
                        nc.tensor.matmul(psf[:], wf_sb[:], uprev[h], start=True, stop=True)
                        psfs.append(psf)
                    for h, (c0, c1) in enumerate(CW):
                        un = upool.tile([NPART, c1 - c0], bf16, tag=f"u{h}")
                        nc.vector.tensor_mul(un[:], psfs[h][0:NPART, :], el_sb[:, k, c0:c1])
                        uprev[h] = un[:]
                    if k == F:
                        ulast = list(uprev)
                # ---- bwd tick σ = k, el time T+1-k (both column halves) ----
                if k <= SB:
                    vns = []
                    for h, (c0, c1) in enumerate(CW):
                        vn = vpool.tile([NPART, c1 - c0], bf16, tag=f"v{h}")
                        src = gprev[h] if gprev_sbuf[h] else gprev[h][0:NPART, :]
                        nc.vector.tensor_mul(vn[:], src, el_sb[:, T + 1 - k, c0:c1])
                        vns.append(vn)
                    for h, (c0, c1) in enumerate(CW):
                        psb = psfpool.tile([MOUT, c1 - c0], f32, tag="ps")
                        nc.tensor.matmul(psb[:], wb_sb[:], vns[h][:], start=True, stop=True)
                        gprev[h] = psb
                        gprev_sbuf[h] = False
                    if k == SB:
                        pb_last = [(gprev[h], gprev_sbuf[h]) for h in range(len(CW))]

            # ---- combine: w = u_F ⊙ γ_S; Zc = per-group colsum of w ----
            accf = fin.tile([G, NCOL], f32, tag="lnu")
            accc = fin.tile([G, NCOL], f32, tag="lnc")
            for h, (c0, c1) in enumerate(CW):
                gl, gl_sbuf = pb_last[h]
                wt = vpool.tile([NPART, c1 - c0], bf16, tag=f"wt{h}")
                nc.vector.tensor_mul(wt[:], gl if gl_sbuf else gl[0:NPART, :], ulast[h])
                psc = psfpool.tile([MOUT, c1 - c0], f32, tag=f"psf{h}")
                nc.tensor.matmul(psc[0:G, :], wcs_sb[:], wt[:], start=True, stop=True)
                nc.scalar.activation(accf[:, c0:c1], ulast[h][NACT:NPART, :], AF.Ln)
                nc.scalar.activation(accc[:, c0:c1], psc[0:G, :], AF.Ln)
            nc.sync.dma_start(out=resf[:], in_=accf[:])
            nc.sync.dma_start(out=resc[:], in_=accc[:])

    nc.compile()
    return nc


def _host_prep(logits, trans, labels, seq_lens):
    logits = np.ascontiguousarray(np.asarray(logits), dtype=np.float32)
    trans = np.asarray(trans, dtype=np.float32)
    labels = np.asarray(labels)
    lens = np.clip(np.asarray(seq_lens), 1, T).astype(np.int64)

    # ---- gold path score (host: index gathers over small inputs) ----
    tmask = np.arange(T)[None, :] < lens[:, None]
    unary = np.take_along_axis(logits, labels[..., None].astype(np.int64), axis=2)[..., 0]
    gp = (unary * tmask).sum(1) + (trans[labels[:, :-1], labels[:, 1:]] * tmask[:, 1:]).sum(1)

    # ---- device inputs: mask every t >= len; pad slice t=T = -inf ----
    lgx = logits.copy()
    lgx[~tmask] = -1e9
    lgx = np.concatenate([lgx, np.full((B, 1, L), -1e9, np.float32)], axis=1)

    el32 = (np.arange(TEX)[None, :] >= lens[:, None]).astype(np.float32)  # [B, 513]

    lg_cores, el32_cores = [], []
    for core in range(NCORES):
        b0 = core * BPC
        lgp = np.full((G, 32, TEX, NCOL), -1e9, np.float32)
        e32 = np.zeros((G, TEX, NCOL), np.float32)
        for g in range(G):
            ncols = NCOL if g < 2 else BPC - 2 * NCOL
            bs = b0 + g * NCOL
            lgp[g, :, :, :ncols] = lgx[bs : bs + ncols].transpose(2, 1, 0)
            e32[g, :, :ncols] = el32[bs : bs + ncols].T
            if ncols < NCOL:  # pad column: dummy len==T sequence, active el = 0
                e32[g, T, ncols:] = 1.0
        lg_cores.append(np.ascontiguousarray(lgp).reshape(NACT, TEX, NCOL))
        el32_cores.append(e32.astype(ml_dtypes.bfloat16))

    # ---- stationary operators ----
    E = np.exp(trans).astype(np.float32)
    Wf = np.zeros((NPART, MOUT), np.float32)
    Wb = np.zeros((NPART, MOUT), np.float32)
    Wcs = np.zeros((NPART, G), np.float32)
    for g in range(G):
        a, sk, cs = 32 * g, NACT + g, NPART + g
        Wf[a : a + 32, a : a + 32] = E
        Wf[a : a + 32, sk] = 1.0
        Wf[sk, sk] = 1.0
        Wf[a : a + 32, cs] = 1.0
        Wf[sk, cs] = 1.0
        Wb[a : a + 32, a : a + 32] = E.T
        Wb[sk, a : a + 32] = 1.0   # sink births β = 1 over all labels
        Wb[sk, sk] = 1.0
        Wb[a : a + 32, cs] = 1.0
        Wb[sk, cs] = 1.0
        Wcs[a : a + 32, g] = 1.0
        Wcs[sk, g] = 1.0
    bf = ml_dtypes.bfloat16
    return gp, lens, lg_cores, el32_cores, Wf.astype(bf), Wb.astype(bf), Wcs.astype(bf)


def _log(msg):
    import time as _t

    print(f"[kernel {_t.strftime('%H:%M:%S')}] {msg}", flush=True)


def kernel(logits, trans, labels, seq_lens):
    global last_result
    from concourse.bass_utils import run_bass_kernel_spmd

    _log("host prep start")
    gp, lens, lg_cores, el32_cores, Wf, Wb, Wcs = _host_prep(
        logits, trans, labels, seq_lens
    )
    _log("host prep done")

    if "nc" not in _prog_cache:
        _prog_cache["nc"] = _build_program()
        _log("program built")
    nc = _prog_cache["nc"]

    in_maps = [
        {
            "lg": lg_cores[i],
            "el32": el32_cores[i],
            "wf": Wf,
            "wbk": Wb,
            "wcs": Wcs,
        }
        for i in range(NCORES)
    ]
    r = run_bass_kernel_spmd(nc, in_maps, core_ids=list(range(NCORES)))
    last_result = r
    _log("device run done")

    # ---- unshard + select sink vs combine per sequence length ----
    devf = np.zeros(B, np.float32)
    devc = np.zeros(B, np.float32)
    for core in range(NCORES):
        rf = r.results[core]["resf"]
        rc = r.results[core]["resc"]
        b0 = core * BPC
        for g in range(G):
            ncols = NCOL if g < 2 else BPC - 2 * NCOL
            devf[b0 + g * NCOL : b0 + g * NCOL + ncols] = rf[g, :ncols]
            devc[b0 + g * NCOL : b0 + g * NCOL + ncols] = rc[g, :ncols]

    dev = np.where(lens <= F, devf, devc)
    logZ = dev + CSHIFT * lens.astype(np.float32)
    return (gp - logZ).astype(np.float32)



# revision 33
# speedup vs baseline: 1.0048x; 1.0048x over previous
"""CRF log-likelihood kernel for Trainium2 (Bass/Tile), 8-core data parallel.

out[b] = gold_path_score(b) - logZ(b)

logZ via exp-domain DP with forward and backward chains meeting at t = F:
  fwd:  u_t   = el_t  ⊙ (Wf^T u_{t-1}),      t = 1..F      (u_0 = el_0)
  bwd:  γ_σ   = Wb^T (el_{T+1-σ} ⊙ γ_{σ-1}), σ = 1..T-F    (γ_0 = sink)
Sequences with len <= F finish inside the fwd chain via an absorbing "sink"
label that captures sum_i u_{len-1}[i] exactly at t == len; longer sequences
use the midpoint identity Z = Σ_j α_F[j]·β_F[j], with the bwd chain's sink
"birthing" β = 1 at each sequence's own end time. The two chains are
independent, so PE matmuls of one overlap DVE multiplies of the other.

Layout per core (128 sequences):
  partitions 0..95 = active labels (3 groups x 32), 96..98 = sink row per
  group; psum rows 99..101 = per-group column sums (ones-columns of the
  stationary operand). columns: b_local = 43*g + c.
Scaling: all emissions carry e^{-CSHIFT}; columns are renormalized by their
column sum mid-chain (factor tracked exactly via ACT-Ln of the applied
multiplier). Host adds CSHIFT*len back and picks sink vs combine per length.
Host also does the gold-path gathers (labels/trans only) and final subtract.
"""

import numpy as np
import ml_dtypes

B, T, L = 1024, 512, 32
NCORES = 8
BPC = B // NCORES        # 128 sequences per core
G = 3                    # label groups per core
NCOL = 43                # columns per group (group 2 uses 42 + 1 pad)
NACT = 96                # active label partitions
NPART = 99               # + 3 sink rows
MOUT = 102               # + 3 colsum rows
CSHIFT = 4.5
TEX = T + 1              # el time slices 0..T
F = 256                  # fwd ticks; bwd ticks = T - F
SB = T - F
RENORM_EVERY = 128


def _el_windows():
    """Graded (t0, n) windows covering [0, TEX), smallest first, alternating
    tail (bwd consumes from t=T down) and head (fwd from t=0 up) so both
    chains can start after ~2 small DMAs instead of waiting out 1MB chunks."""
    sizes = [8, 16, 32, 64, 96]
    head, tail = [], []
    lo, hi = 0, TEX
    for s in sizes:
        tail.append((hi - s, s)); hi -= s
        head.append((lo, s)); lo += s
    # remainder split once more (tail gets the first share)
    rem = hi - lo
    a = rem // 2
    tail.append((hi - a, a)); hi -= a
    head.append((lo, hi - lo))
    order = []
    for tl, hd in zip(tail, head):
        order.append(tl); order.append(hd)
    return order


EL_WINDOWS = _el_windows()
STAGE_MAX = max(n for _, n in EL_WINDOWS)

_prog_cache = {}
last_result = None       # BassKernelResults of the most recent run (for test.py)


def _build_program():
    import concourse.bacc as bacc
    import concourse.tile as tile
    from concourse import mybir

    f32 = mybir.dt.float32
    bf16 = mybir.dt.bfloat16
    AF = mybir.ActivationFunctionType

    nc = bacc.Bacc("TRN2", target_bir_lowering=False, debug=False, num_devices=NCORES)
    lg = nc.dram_tensor("lg", [NACT, TEX, NCOL], f32, kind="ExternalInput")
    el32 = nc.dram_tensor("el32", [G, TEX, NCOL], bf16, kind="ExternalInput")
    wf = nc.dram_tensor("wf", [NPART, MOUT], bf16, kind="ExternalInput")
    wbk = nc.dram_tensor("wbk", [NPART, MOUT], bf16, kind="ExternalInput")
    wcs = nc.dram_tensor("wcs", [NPART, G], bf16, kind="ExternalInput")
    resf = nc.dram_tensor("resf", [G, NCOL], f32, kind="ExternalOutput")
    resc = nc.dram_tensor("resc", [G, NCOL], f32, kind="ExternalOutput")

    with tile.TileContext(nc) as tc:
        with (
            tc.tile_pool(name="big", bufs=1) as big,
            tc.tile_pool(name="stage", bufs=3) as stage_p,
            tc.tile_pool(name="consts", bufs=1) as consts,
            tc.tile_pool(name="u", bufs=3) as upool,
            tc.tile_pool(name="v", bufs=3) as vpool,
            tc.tile_pool(name="fin", bufs=1) as fin,
            tc.tile_pool(name="psf", bufs=4, space="PSUM") as psfpool,
        ):
            el_sb = big.tile([NPART, TEX, NCOL], bf16)
            wf_sb = consts.tile([NPART, MOUT], bf16)
            wb_sb = consts.tile([NPART, MOUT], bf16)
            wcs_sb = consts.tile([NPART, G], bf16)
            biasc = consts.tile([128, 1], f32)
            g0 = consts.tile([NPART, NCOL], bf16)
            nc.vector.memset(biasc[:], -CSHIFT)
            nc.vector.memset(g0[:], 0.0)
            nc.vector.memset(g0[NACT:NPART, :], 1.0)

            # warm the ACT Exp/Ln tables while the first DMAs are in flight
            warm = consts.tile([1, 2], f32)
            nc.scalar.activation(warm[:, 0:1], biasc[0:1, :], AF.Exp)
            nc.scalar.activation(warm[:, 1:2], warm[:, 0:1], AF.Ln)

            nc.sync.dma_start(out=wf_sb[:], in_=wf[:])
            nc.sync.dma_start(out=wb_sb[:], in_=wbk[:])
            nc.sync.dma_start(out=wcs_sb[:], in_=wcs[:])
            # active rows: stage raw logits, bulk-exp into el_sb.
            # graded windows, alternating ends: bwd consumes from t=T down.
            # sink rows (partitions 96..98) stream in the same windows so no
            # monolithic descriptor hogs a DMA engine ahead of the first chunks.
            for t0, n in EL_WINDOWS:
                st = stage_p.tile([NACT, STAGE_MAX, NCOL], f32, tag="stage")
                nc.sync.dma_start(out=st[:, 0:n, :], in_=lg[:, t0 : t0 + n, :])
                nc.sync.dma_start(
                    out=el_sb[NACT:NPART, t0 : t0 + n, :], in_=el32[:, t0 : t0 + n, :]
                )
                nc.scalar.activation(
                    el_sb[0:NACT, t0 : t0 + n, :],
                    st[:, 0:n, :],
                    AF.Exp,
                    bias=biasc[0:NACT, :],
                )

            # Four independent latency lanes: fwd/bwd x column halves.
            # No renorm: CSHIFT ~ log(L * E[e^trans] * E[e^logit]) makes the
            # expected per-tick log-drift ~0; the +-4 sigma random walk over
            # 256 ticks stays well inside bf16/f32 exponent range.
            CW = [(0, NCOL)]
            uprev = [el_sb[:, 0, c0:c1] for c0, c1 in CW]
            gprev = [g0[:, c0:c1] for c0, c1 in CW]
            gprev_sbuf = [True, True]
            ulast = [None, None]
            pb_last = [None, None]
            for k in range(1, max(F, SB) + 1):
                # ---- fwd tick t = k (both column halves) ----
                if k <= F:
                    psfs = []
                    for h, (c0, c1) in enumerate(CW):
                        psf = psfpool.tile([MOUT, c1 - c0], f32, tag="ps")
                        nc.tensor.matmul(psf[:], wf_sb[:], uprev[h], start=True, stop=True)
                        psfs.append(psf)
                    for h, (c0, c1) in enumerate(CW):
                        un = upool.tile([NPART, c1 - c0], bf16, tag=f"u{h}")
                        nc.vector.tensor_mul(un[:], psfs[h][0:NPART, :], el_sb[:, k, c0:c1])
                        uprev[h] = un[:]
                    if k == F:
                        ulast = list(uprev)
                # ---- bwd tick σ = k, el time T+1-k (both column halves) ----
                if k <= SB:
                    vns = []
                    for h, (c0, c1) in enumerate(CW):
                        vn = vpool.tile([NPART, c1 - c0], bf16, tag=f"v{h}")
                        src = gprev[h] if gprev_sbuf[h] else gprev[h][0:NPART, :]
                        nc.vector.tensor_mul(vn[:], src, el_sb[:, T + 1 - k, c0:c1])
                        vns.append(vn)
                    for h, (c0, c1) in enumerate(CW):
                        psb = psfpool.tile([MOUT, c1 - c0], f32, tag="ps")
                        nc.tensor.matmul(psb[:], wb_sb[:], vns[h][:], start=True, stop=True)
                        gprev[h] = psb
                        gprev_sbuf[h] = False
                    if k == SB:
                        pb_last = [(gprev[h], gprev_sbuf[h]) for h in range(len(CW))]

            # ---- combine: w = u_F ⊙ γ_S; Zc = per-group colsum of w ----
            accf = fin.tile([G, NCOL], f32, tag="lnu")
            accc = fin.tile([G, NCOL], f32, tag="lnc")
            for h, (c0, c1) in enumerate(CW):
                gl, gl_sbuf = pb_last[h]
                wt = vpool.tile([NPART, c1 - c0], bf16, tag=f"wt{h}")
                nc.vector.tensor_mul(wt[:], gl if gl_sbuf else gl[0:NPART, :], ulast[h])
                psc = psfpool.tile([MOUT, c1 - c0], f32, tag=f"psf{h}")
                nc.tensor.matmul(psc[0:G, :], wcs_sb[:], wt[:], start=True, stop=True)
                nc.scalar.activation(accf[:, c0:c1], ulast[h][NACT:NPART, :], AF.Ln)
                nc.scalar.activation(accc[:, c0:c1], psc[0:G, :], AF.Ln)
            nc.sync.dma_start(out=resf[:], in_=accf[:])
            nc.sync.dma_start(out=resc[:], in_=accc[:])

    nc.compile()
    return nc


def _host_prep(logits, trans, labels, seq_lens):
    logits = np.ascontiguousarray(np.asarray(logits), dtype=np.float32)
    trans = np.asarray(trans, dtype=np.float32)
    labels = np.asarray(labels)
    lens = np.clip(np.asarray(seq_lens), 1, T).astype(np.int64)

    # ---- gold path score (host: index gathers over small inputs) ----
    tmask = np.arange(T)[None, :] < lens[:, None]
    unary = np.take_along_axis(logits, labels[..., None].astype(np.int64), axis=2)[..., 0]
    gp = (unary * tmask).sum(1) + (trans[labels[:, :-1], labels[:, 1:]] * tmask[:, 1:]).sum(1)

    # ---- device inputs: mask every t >= len; pad slice t=T = -inf ----
    lgx = logits.copy()
    lgx[~tmask] = -1e9
    lgx = np.concatenate([lgx, np.full((B, 1, L), -1e9, np.float32)], axis=1)

    el32 = (np.arange(TEX)[None, :] >= lens[:, None]).astype(np.float32)  # [B, 513]

    lg_cores, el32_cores = [], []
    for core in range(NCORES):
        b0 = core * BPC
        lgp = np.full((G, 32, TEX, NCOL), -1e9, np.float32)
        e32 = np.zeros((G, TEX, NCOL), np.float32)
        for g in range(G):
            ncols = NCOL if g < 2 else BPC - 2 * NCOL
            bs = b0 + g * NCOL
            lgp[g, :, :, :ncols] = lgx[bs : bs + ncols].transpose(2, 1, 0)
            e32[g, :, :ncols] = el32[bs : bs + ncols].T
            if ncols < NCOL:  # pad column: dummy len==T sequence, active el = 0
                e32[g, T, ncols:] = 1.0
        lg_cores.append(np.ascontiguousarray(lgp).reshape(NACT, TEX, NCOL))
        el32_cores.append(e32.astype(ml_dtypes.bfloat16))

    # ---- stationary operators ----
    E = np.exp(trans).astype(np.float32)
    Wf = np.zeros((NPART, MOUT), np.float32)
    Wb = np.zeros((NPART, MOUT), np.float32)
    Wcs = np.zeros((NPART, G), np.float32)
    for g in range(G):
        a, sk, cs = 32 * g, NACT + g, NPART + g
        Wf[a : a + 32, a : a + 32] = E
        Wf[a : a + 32, sk] = 1.0
        Wf[sk, sk] = 1.0
        Wf[a : a + 32, cs] = 1.0
        Wf[sk, cs] = 1.0
        Wb[a : a + 32, a : a + 32] = E.T
        Wb[sk, a : a + 32] = 1.0   # sink births β = 1 over all labels
        Wb[sk, sk] = 1.0
        Wb[a : a + 32, cs] = 1.0
        Wb[sk, cs] = 1.0
        Wcs[a : a + 32, g] = 1.0
        Wcs[sk, g] = 1.0
    bf = ml_dtypes.bfloat16
    return gp, lens, lg_cores, el32_cores, Wf.astype(bf), Wb.astype(bf), Wcs.astype(bf)


def _log(msg):
    import time as _t

    print(f"[kernel {_t.strftime('%H:%M:%S')}] {msg}", flush=True)


def kernel(logits, trans, labels, seq_lens):
    global last_result
    from concourse.bass_utils import run_bass_kernel_spmd

    _log("host prep start")
    gp, lens, lg_cores, el32_cores, Wf, Wb, Wcs = _host_prep(
        logits, trans, labels, seq_lens
    )
    _log("host prep done")

    if "nc" not in _prog_cache:
        _prog_cache["nc"] = _build_program()
        _log("program built")
    nc = _prog_cache["nc"]

    in_maps = [
        {
            "lg": lg_cores[i],
            "el32": el32_cores[i],
            "wf": Wf,
            "wbk": Wb,
            "wcs": Wcs,
        }
        for i in range(NCORES)
    ]
    r = run_bass_kernel_spmd(nc, in_maps, core_ids=list(range(NCORES)))
    last_result = r
    _log("device run done")

    # ---- unshard + select sink vs combine per sequence length ----
    devf = np.zeros(B, np.float32)
    devc = np.zeros(B, np.float32)
    for core in range(NCORES):
        rf = r.results[core]["resf"]
        rc = r.results[core]["resc"]
        b0 = core * BPC
        for g in range(G):
            ncols = NCOL if g < 2 else BPC - 2 * NCOL
            devf[b0 + g * NCOL : b0 + g * NCOL + ncols] = rf[g, :ncols]
            devc[b0 + g * NCOL : b0 + g * NCOL + ncols] = rc[g, :ncols]

    dev = np.where(lens <= F, devf, devc)
    logZ = dev + CSHIFT * lens.astype(np.float32)
    return (gp - logZ).astype(np.float32)



# revision 34
# speedup vs baseline: 1.0233x; 1.0184x over previous
"""CRF log-likelihood kernel for Trainium2 (Bass/Tile), 8-core data parallel.

out[b] = gold_path_score(b) - logZ(b)

logZ via exp-domain DP with forward and backward chains meeting at t = F:
  fwd:  u_t   = el_t  ⊙ (Wf^T u_{t-1}),      t = 1..F      (u_0 = el_0)
  bwd:  γ_σ   = Wb^T (el_{T+1-σ} ⊙ γ_{σ-1}), σ = 1..T-F    (γ_0 = sink)
Sequences with len <= F finish inside the fwd chain via an absorbing "sink"
label that captures sum_i u_{len-1}[i] exactly at t == len; longer sequences
use the midpoint identity Z = Σ_j α_F[j]·β_F[j], with the bwd chain's sink
"birthing" β = 1 at each sequence's own end time. The two chains are
independent, so PE matmuls of one overlap DVE multiplies of the other.

Layout per core (128 sequences):
  partitions 0..95 = active labels (3 groups x 32), 96..98 = sink row per
  group; psum rows 99..101 = per-group column sums (ones-columns of the
  stationary operand). columns: b_local = 43*g + c.
Scaling: all emissions carry e^{-CSHIFT}; columns are renormalized by their
column sum mid-chain (factor tracked exactly via ACT-Ln of the applied
multiplier). Host adds CSHIFT*len back and picks sink vs combine per length.
Host also does the gold-path gathers (labels/trans only) and final subtract.
"""

import numpy as np
import ml_dtypes

B, T, L = 1024, 512, 32
NCORES = 8
BPC = B // NCORES        # 128 sequences per core
G = 3                    # label groups per core
NCOL = 43                # columns per group (group 2 uses 42 + 1 pad)
NACT = 96                # active label partitions
NPART = 99               # + 3 sink rows
MOUT = 102               # + 3 colsum rows
CSHIFT = 4.5
TEX = T + 1              # el time slices 0..T
F = 256                  # fwd ticks; bwd ticks = T - F
SB = T - F
RENORM_EVERY = 128


def _el_windows():
    """Graded (t0, n) windows covering [0, TEX), smallest first, alternating
    tail (bwd consumes from t=T down) and head (fwd from t=0 up) so both
    chains can start after ~2 small DMAs instead of waiting out 1MB chunks."""
    sizes = [8, 16, 32, 64, 96]
    head, tail = [], []
    lo, hi = 0, TEX
    for s in sizes:
        tail.append((hi - s, s)); hi -= s
        head.append((lo, s)); lo += s
    # remainder split once more (tail gets the first share)
    rem = hi - lo
    a = rem // 2
    tail.append((hi - a, a)); hi -= a
    head.append((lo, hi - lo))
    order = []
    for tl, hd in zip(tail, head):
        order.append(tl); order.append(hd)
    return order


EL_WINDOWS = _el_windows()
STAGE_MAX = max(n for _, n in EL_WINDOWS)

_prog_cache = {}
last_result = None       # BassKernelResults of the most recent run (for test.py)


def _build_program():
    import concourse.bacc as bacc
    import concourse.tile as tile
    from concourse import mybir

    f32 = mybir.dt.float32
    bf16 = mybir.dt.bfloat16
    AF = mybir.ActivationFunctionType

    nc = bacc.Bacc("TRN2", target_bir_lowering=False, debug=False, num_devices=NCORES)
    lg = nc.dram_tensor("lg", [NPART, TEX, NCOL], f32, kind="ExternalInput")
    wf = nc.dram_tensor("wf", [NPART, MOUT], bf16, kind="ExternalInput")
    wbk = nc.dram_tensor("wbk", [NPART, MOUT], bf16, kind="ExternalInput")
    wcs = nc.dram_tensor("wcs", [NPART, G], bf16, kind="ExternalInput")
    resf = nc.dram_tensor("resf", [G, NCOL], f32, kind="ExternalOutput")
    resc = nc.dram_tensor("resc", [G, NCOL], f32, kind="ExternalOutput")

    with tile.TileContext(nc) as tc:
        with (
            tc.tile_pool(name="big", bufs=1) as big,
            tc.tile_pool(name="stage", bufs=4) as stage_p,
            tc.tile_pool(name="consts", bufs=1) as consts,
            tc.tile_pool(name="u", bufs=3) as upool,
            tc.tile_pool(name="v", bufs=3) as vpool,
            tc.tile_pool(name="fin", bufs=1) as fin,
            tc.tile_pool(name="psf", bufs=4, space="PSUM") as psfpool,
        ):
            el_sb = big.tile([NPART, TEX, NCOL], bf16)
            wf_sb = consts.tile([NPART, MOUT], bf16)
            wb_sb = consts.tile([NPART, MOUT], bf16)
            wcs_sb = consts.tile([NPART, G], bf16)
            biasc = consts.tile([128, 1], f32)
            g0 = consts.tile([NPART, NCOL], bf16)
            nc.vector.memset(biasc[:], -CSHIFT)
            nc.vector.memset(g0[:], 0.0)
            nc.vector.memset(g0[NACT:NPART, :], 1.0)

            # warm the ACT Exp/Ln tables while the first DMAs are in flight
            warm = consts.tile([1, 2], f32)
            nc.scalar.activation(warm[:, 0:1], biasc[0:1, :], AF.Exp)
            nc.scalar.activation(warm[:, 1:2], warm[:, 0:1], AF.Ln)

            # stage raw logits (sink-indicator rows ride along, CSHIFT-
            # compensated so exp yields exactly 0/1), bulk-exp into el_sb.
            # graded windows alternating ends; ONE dma per window keeps the
            # Sync-engine enqueue queue short so the chains start early.
            # weights enqueue after the first window pair, wcs after all.
            for wi, (t0, n) in enumerate(EL_WINDOWS):
                st = stage_p.tile([NPART, STAGE_MAX, NCOL], f32, tag="stage")
                nc.sync.dma_start(out=st[:, 0:n, :], in_=lg[:, t0 : t0 + n, :])
                nc.scalar.activation(
                    el_sb[:, t0 : t0 + n, :],
                    st[:, 0:n, :],
                    AF.Exp,
                    bias=biasc[0:NPART, :],
                )
                if wi == 1:
                    nc.sync.dma_start(out=wf_sb[:], in_=wf[:])
                    nc.sync.dma_start(out=wb_sb[:], in_=wbk[:])
            nc.sync.dma_start(out=wcs_sb[:], in_=wcs[:])

            # Four independent latency lanes: fwd/bwd x column halves.
            # No renorm: CSHIFT ~ log(L * E[e^trans] * E[e^logit]) makes the
            # expected per-tick log-drift ~0; the +-4 sigma random walk over
            # 256 ticks stays well inside bf16/f32 exponent range.
            CW = [(0, NCOL)]
            uprev = [el_sb[:, 0, c0:c1] for c0, c1 in CW]
            gprev = [g0[:, c0:c1] for c0, c1 in CW]
            gprev_sbuf = [True, True]
            ulast = [None, None]
            pb_last = [None, None]
            for k in range(1, max(F, SB) + 1):
                # ---- fwd tick t = k (both column halves) ----
                if k <= F:
                    psfs = []
                    for h, (c0, c1) in enumerate(CW):
                        psf = psfpool.tile([MOUT, c1 - c0], f32, tag="ps")
                        nc.tensor.matmul(psf[:], wf_sb[:], uprev[h], start=True, stop=True)
                        psfs.append(psf)
                    for h, (c0, c1) in enumerate(CW):
                        un = upool.tile([NPART, c1 - c0], bf16, tag=f"u{h}")
                        nc.vector.tensor_mul(un[:], psfs[h][0:NPART, :], el_sb[:, k, c0:c1])
                        uprev[h] = un[:]
                    if k == F:
                        ulast = list(uprev)
                # ---- bwd tick σ = k, el time T+1-k (both column halves) ----
                if k <= SB:
                    vns = []
                    for h, (c0, c1) in enumerate(CW):
                        vn = vpool.tile([NPART, c1 - c0], bf16, tag=f"v{h}")
                        src = gprev[h] if gprev_sbuf[h] else gprev[h][0:NPART, :]
                        nc.vector.tensor_mul(vn[:], src, el_sb[:, T + 1 - k, c0:c1])
                        vns.append(vn)
                    for h, (c0, c1) in enumerate(CW):
                        psb = psfpool.tile([MOUT, c1 - c0], f32, tag="ps")
                        nc.tensor.matmul(psb[:], wb_sb[:], vns[h][:], start=True, stop=True)
                        gprev[h] = psb
                        gprev_sbuf[h] = False
                    if k == SB:
                        pb_last = [(gprev[h], gprev_sbuf[h]) for h in range(len(CW))]

            # ---- combine: w = u_F ⊙ γ_S; Zc = per-group colsum of w ----
            accf = fin.tile([G, NCOL], f32, tag="lnu")
            accc = fin.tile([G, NCOL], f32, tag="lnc")
            for h, (c0, c1) in enumerate(CW):
                gl, gl_sbuf = pb_last[h]
                wt = vpool.tile([NPART, c1 - c0], bf16, tag=f"wt{h}")
                nc.vector.tensor_mul(wt[:], gl if gl_sbuf else gl[0:NPART, :], ulast[h])
                psc = psfpool.tile([MOUT, c1 - c0], f32, tag=f"psf{h}")
                nc.tensor.matmul(psc[0:G, :], wcs_sb[:], wt[:], start=True, stop=True)
                nc.scalar.activation(accf[:, c0:c1], ulast[h][NACT:NPART, :], AF.Ln)
                nc.scalar.activation(accc[:, c0:c1], psc[0:G, :], AF.Ln)
            nc.sync.dma_start(out=resf[:], in_=accf[:])
            nc.sync.dma_start(out=resc[:], in_=accc[:])

    nc.compile()
    return nc


def _host_prep(logits, trans, labels, seq_lens):
    logits = np.ascontiguousarray(np.asarray(logits), dtype=np.float32)
    trans = np.asarray(trans, dtype=np.float32)
    labels = np.asarray(labels)
    lens = np.clip(np.asarray(seq_lens), 1, T).astype(np.int64)

    # ---- gold path score (host: index gathers over small inputs) ----
    tmask = np.arange(T)[None, :] < lens[:, None]
    unary = np.take_along_axis(logits, labels[..., None].astype(np.int64), axis=2)[..., 0]
    gp = (unary * tmask).sum(1) + (trans[labels[:, :-1], labels[:, 1:]] * tmask[:, 1:]).sum(1)

    # ---- device inputs: mask every t >= len; pad slice t=T = -inf ----
    lgx = logits.copy()
    lgx[~tmask] = -1e9
    lgx = np.concatenate([lgx, np.full((B, 1, L), -1e9, np.float32)], axis=1)

    # sink-indicator rows in log space, pre-compensated for the exp bias so
    # the on-device exp(x - CSHIFT) yields exactly 1.0 / 0.0
    el32 = np.where(np.arange(TEX)[None, :] >= lens[:, None], CSHIFT, -1e9).astype(
        np.float32
    )  # [B, 513]

    lg_cores = []
    for core in range(NCORES):
        b0 = core * BPC
        lgp = np.full((NPART, TEX, NCOL), -1e9, np.float32)
        for g in range(G):
            ncols = NCOL if g < 2 else BPC - 2 * NCOL
            bs = b0 + g * NCOL
            lgp[32 * g : 32 * g + 32, :, :ncols] = lgx[bs : bs + ncols].transpose(2, 1, 0)
            lgp[NACT + g, :, :ncols] = el32[bs : bs + ncols].T
            if ncols < NCOL:  # pad column: dummy len==T sequence, active el = 0
                lgp[NACT + g, T, ncols:] = CSHIFT
        lg_cores.append(np.ascontiguousarray(lgp))

    # ---- stationary operators ----
    E = np.exp(trans).astype(np.float32)
    Wf = np.zeros((NPART, MOUT), np.float32)
    Wb = np.zeros((NPART, MOUT), np.float32)
    Wcs = np.zeros((NPART, G), np.float32)
    for g in range(G):
        a, sk, cs = 32 * g, NACT + g, NPART + g
        Wf[a : a + 32, a : a + 32] = E
        Wf[a : a + 32, sk] = 1.0
        Wf[sk, sk] = 1.0
        Wf[a : a + 32, cs] = 1.0
        Wf[sk, cs] = 1.0
        Wb[a : a + 32, a : a + 32] = E.T
        Wb[sk, a : a + 32] = 1.0   # sink births β = 1 over all labels
        Wb[sk, sk] = 1.0
        Wb[a : a + 32, cs] = 1.0
        Wb[sk, cs] = 1.0
        Wcs[a : a + 32, g] = 1.0
        Wcs[sk, g] = 1.0
    bf = ml_dtypes.bfloat16
    return gp, lens, lg_cores, Wf.astype(bf), Wb.astype(bf), Wcs.astype(bf)


def _log(msg):
    import time as _t

    print(f"[kernel {_t.strftime('%H:%M:%S')}] {msg}", flush=True)


def kernel(logits, trans, labels, seq_lens):
    global last_result
    from concourse.bass_utils import run_bass_kernel_spmd

    _log("host prep start")
    gp, lens, lg_cores, Wf, Wb, Wcs = _host_prep(
        logits, trans, labels, seq_lens
    )
    _log("host prep done")

    if "nc" not in _prog_cache:
        _prog_cache["nc"] = _build_program()
        _log("program built")
    nc = _prog_cache["nc"]

    in_maps = [
        {"lg": lg_cores[i], "wf": Wf, "wbk": Wb, "wcs": Wcs}
        for i in range(NCORES)
    ]
    r = run_bass_kernel_spmd(nc, in_maps, core_ids=list(range(NCORES)))
    last_result = r
    _log("device run done")

    # ---- unshard + select sink vs combine per sequence length ----
    devf = np.zeros(B, np.float32)
    devc = np.zeros(B, np.float32)
    for core in range(NCORES):
        rf = r.results[core]["resf"]
        rc = r.results[core]["resc"]
        b0 = core * BPC
        for g in range(G):
            ncols = NCOL if g < 2 else BPC - 2 * NCOL
            devf[b0 + g * NCOL : b0 + g * NCOL + ncols] = rf[g, :ncols]
            devc[b0 + g * NCOL : b0 + g * NCOL + ncols] = rc[g, :ncols]

    dev = np.where(lens <= F, devf, devc)
    logZ = dev + CSHIFT * lens.astype(np.float32)
    return (gp - logZ).astype(np.float32)

